# revision 3
# baseline (speedup 1.0000x reference)
"""Trainium2 Bass kernel for the CGC (Customized Gate Control) MoE routing module.

Contract: kernel(**inputs) takes the FULL unsharded inputs (numpy/jax arrays)
and returns the FULL output [5, 16384, 256] float32.

Strategy:
  - Data-parallel over batch across 8 NeuronCores (2048 rows/core).
  - Host-side prep: per-core x slices are fed pre-transposed [DIN, B_c] so the
    contraction dim lands on SBUF partitions with fully contiguous DMAs (no
    on-device transposes). Expert weights are replicated and packed [DIN, E*H].
  - Expert biases are accumulated into PSUM by a K=32 matmul against a one-hot
    "ones" row, so the ScalarE eviction is a pure relu (and can fold the first
    gate-scaled combine term via its per-partition scale operand).
  - The double-softmax gate mask is known from sim_domain at trace time, so the
    kernel is compile-specialized: masked shared-expert terms are not emitted.
"""

import sys

sys.path.insert(0, "/opt/trn_rl_repo")

import numpy as np

D_NUM = 4
N_ES = 2
N_SH = 4
DIN = 512
H = 256
B = 16384
N_CORES = 8
BC = B // N_CORES          # 2048 rows per core
KC = DIN // 128            # 4 contraction chunks
GRP = 2                    # batch tiles (of 128 rows) per group
NG = BC // (128 * GRP)     # 8 groups per core

# bias row layout: [spec d*512 .. | shared 1024 | gate 4*6 | gate_sh 12]
OFF_SH = D_NUM * 512       # 2048
OFF_G = OFF_SH + N_SH * H  # 3072
OFF_GS = OFF_G + D_NUM * (N_ES + N_SH)  # 3096
NB = OFF_GS + D_NUM * N_ES + N_SH       # 3108

USE_GPS = False  # TensorScalarPtr is not a valid Pool-engine op on TRN2

_BUILD_CACHE = {}


def _build(allowed):
    """Trace + compile the per-core kernel, specialized on the allowed
    shared-expert sets (from sim_domain). Returns the compiled Bacc."""
    import concourse.bacc as bacc
    import concourse.bass as bass
    import concourse.mybir as mybir
    import concourse.tile as tile

    f32 = mybir.dt.float32
    Alu = mybir.AluOpType
    Act = mybir.ActivationFunctionType
    Ax = mybir.AxisListType

    nc = bacc.Bacc(None, target_bir_lowering=False, debug=False)

    xt = nc.declare_dram_parameter("xt", [5, DIN, BC], f32, isOutput=False)
    wsp = nc.declare_dram_parameter("wsp", [D_NUM, DIN, N_ES * H], f32, isOutput=False)
    wsh = nc.declare_dram_parameter("wsh", [DIN, N_SH * H], f32, isOutput=False)
    wg = nc.declare_dram_parameter("wg", [DIN, D_NUM * 6], f32, isOutput=False)
    wgs = nc.declare_dram_parameter("wgs", [DIN, 12], f32, isOutput=False)
    bias = nc.declare_dram_parameter("bias", [32, NB], f32, isOutput=False)
    ones = nc.declare_dram_parameter("ones", [32, 128], f32, isOutput=False)
    bmask = nc.declare_dram_parameter("bmask", [128, D_NUM, GRP, 6], f32, isOutput=False)
    out = nc.declare_dram_parameter("out", [5, BC, H], f32, isOutput=True)

    with tile.TileContext(nc) as tc:
        with (
            tc.tile_pool(name="wpool", bufs=1) as wp,
            tc.tile_pool(name="xpool", bufs=2) as xp,
            tc.tile_pool(name="ogpool", bufs=2) as ogp,
            tc.tile_pool(name="specpool", bufs=2) as spp,
            tc.tile_pool(name="shpool", bufs=2) as shp,
            tc.tile_pool(name="smpool", bufs=8) as sp,
            tc.tile_pool(name="accpool", bufs=4) as ap_,
            tc.tile_pool(name="pbig", bufs=3, space=bass.MemorySpace.PSUM) as pb,
            tc.tile_pool(name="pgate", bufs=5, space=bass.MemorySpace.PSUM) as pg,
        ):
            # ---- persistent weights ----
            wsp_sb = wp.tile([128, D_NUM, KC, N_ES * H], f32, tag="wsp")
            nc.sync.dma_start(wsp_sb[:], wsp.rearrange("d (c p) n -> p d c n", p=128))
            wsh_sb = wp.tile([128, KC, N_SH * H], f32, tag="wsh")
            nc.sync.dma_start(wsh_sb[:], wsh.rearrange("(c p) n -> p c n", p=128))
            wg_sb = wp.tile([128, KC, D_NUM * 6], f32, tag="wg")
            nc.sync.dma_start(wg_sb[:], wg.rearrange("(c p) n -> p c n", p=128))
            wgs_sb = wp.tile([128, KC, 12], f32, tag="wgs")
            nc.sync.dma_start(wgs_sb[:], wgs.rearrange("(c p) n -> p c n", p=128))
            bias_sb = wp.tile([32, NB], f32, tag="bias")
            nc.sync.dma_start(bias_sb[:], bias[:])
            ones_sb = wp.tile([32, 128], f32, tag="ones")
            nc.sync.dma_start(ones_sb[:], ones[:])
            bmask_sb = wp.tile([128, D_NUM, GRP, 6], f32, tag="bmask")
            nc.sync.dma_start(bmask_sb[:], bmask[:])

            for g in range(NG):
                j0 = g * (GRP * 128)
                xtg = xp.tile([128, 5, KC, GRP * 128], f32, tag="xtg", name=f"xtg{g}")
                nc.sync.dma_start(
                    xtg[:],
                    xt[:, :, j0 : j0 + GRP * 128].rearrange("i (c p) j -> p i c j", p=128),
                )
                og = ogp.tile([128, 5, GRP, H], f32, tag="og", name=f"og{g}")

                # ---- gate phase (PE): all gate logits for the group ----
                pgd = []
                for d in range(D_NUM):
                    pgt = pg.tile([128, GRP, 6], f32, tag="pg", name=f"pg{g}_{d}")
                    pgd.append(pgt)
                    for t in range(GRP):
                        nc.tensor.matmul(
                            pgt[:, t, :], ones_sb[:, :],
                            bias_sb[:, OFF_G + 6 * d : OFF_G + 6 * d + 6],
                            start=True, stop=False,
                        )
                        for c in range(KC):
                            nc.tensor.matmul(
                                pgt[:, t, :],
                                xtg[:, d, c, t * 128 : (t + 1) * 128],
                                wg_sb[:, c, 6 * d : 6 * d + 6],
                                start=False, stop=(c == KC - 1),
                            )
                pgs = pg.tile([128, GRP, 12], f32, tag="pg", name=f"pgs{g}")
                for t in range(GRP):
                    nc.tensor.matmul(
                        pgs[:, t, :], ones_sb[:, :], bias_sb[:, OFF_GS : OFF_GS + 12],
                        start=True, stop=False,
                    )
                    for c in range(KC):
                        nc.tensor.matmul(
                            pgs[:, t, :],
                            xtg[:, 4, c, t * 128 : (t + 1) * 128],
                            wgs_sb[:, c, :],
                            start=False, stop=(c == KC - 1),
                        )

                # ---- softmax phase (ACT exp + DVE reductions) ----
                g2d = []
                for d in range(D_NUM):
                    e1 = sp.tile([128, GRP, 6], f32, tag="e1", name=f"e1_{g}_{d}")
                    nc.scalar.activation(e1[:], pgd[d][:], Act.Exp)
                    s1 = sp.tile([128, GRP], f32, tag="s1", name=f"s1_{g}_{d}")
                    nc.vector.tensor_reduce(s1[:], e1[:], axis=Ax.X, op=Alu.add)
                    r1 = sp.tile([128, GRP], f32, tag="r1", name=f"r1_{g}_{d}")
                    nc.vector.reciprocal(r1[:], s1[:])
                    gn = sp.tile([128, GRP, 6], f32, tag="gn", name=f"gn_{g}_{d}")
                    for t in range(GRP):
                        nc.vector.tensor_scalar_mul(gn[:, t, :], e1[:, t, :], r1[:, t : t + 1])
                    e2 = sp.tile([128, GRP, 6], f32, tag="e2", name=f"e2_{g}_{d}")
                    nc.scalar.activation(e2[:], gn[:], Act.Exp)
                    e2m = sp.tile([128, GRP, 6], f32, tag="e2m", name=f"e2m_{g}_{d}")
                    nc.vector.tensor_tensor(e2m[:], e2[:], bmask_sb[:, d], Alu.mult)
                    s2 = sp.tile([128, GRP], f32, tag="s2", name=f"s2_{g}_{d}")
                    nc.vector.tensor_reduce(s2[:], e2m[:], axis=Ax.X, op=Alu.add)
                    r2 = sp.tile([128, GRP], f32, tag="r2", name=f"r2_{g}_{d}")
                    nc.vector.reciprocal(r2[:], s2[:])
                    g2 = sp.tile([128, GRP, 6], f32, tag="g2", name=f"g2_{g}_{d}")
                    for t in range(GRP):
                        nc.vector.tensor_scalar_mul(g2[:, t, :], e2m[:, t, :], r2[:, t : t + 1])
                    g2d.append(g2)
                egs = sp.tile([128, GRP, 12], f32, tag="egs", name=f"egs{g}")
                nc.scalar.activation(egs[:], pgs[:], Act.Exp)
                sgs = sp.tile([128, GRP], f32, tag="sgs", name=f"sgs{g}")
                nc.vector.tensor_reduce(sgs[:], egs[:], axis=Ax.X, op=Alu.add)
                rgs = sp.tile([128, GRP], f32, tag="rgs", name=f"rgs{g}")
                nc.vector.reciprocal(rgs[:], sgs[:])
                gsn = sp.tile([128, GRP, 12], f32, tag="gsn", name=f"gsn{g}")
                for t in range(GRP):
                    nc.vector.tensor_scalar_mul(gsn[:, t, :], egs[:, t, :], rgs[:, t : t + 1])

                # ---- expert + combine phase, per batch tile ----
                for t in range(GRP):
                    spec = spp.tile([128, D_NUM, N_ES * H], f32, tag="spec", name=f"spec{g}_{t}")
                    sh = shp.tile([128, N_SH * H], f32, tag="sh", name=f"sh{g}_{t}")

                    # shared experts (2 PSUM banks)
                    ps_a = pb.tile([128, 512], f32, tag="pb", name=f"psa{g}_{t}")
                    nc.tensor.matmul(ps_a[:], ones_sb[:, :], bias_sb[:, OFF_SH : OFF_SH + 512],
                                     start=True, stop=False)
                    for c in range(KC):
                        nc.tensor.matmul(ps_a[:], xtg[:, 4, c, t * 128 : (t + 1) * 128],
                                         wsh_sb[:, c, 0:512], start=False, stop=(c == KC - 1))
                    ps_b = pb.tile([128, 512], f32, tag="pb", name=f"psb{g}_{t}")
                    nc.tensor.matmul(ps_b[:], ones_sb[:, :], bias_sb[:, OFF_SH + 512 : OFF_SH + 1024],
                                     start=True, stop=False)
                    for c in range(KC):
                        nc.tensor.matmul(ps_b[:], xtg[:, 4, c, t * 128 : (t + 1) * 128],
                                         wsh_sb[:, c, 512:1024], start=False, stop=(c == KC - 1))

                    og_s = og[:, 4, t, :]
                    # out_sh first term: gs[8] * relu(shared expert 0)
                    nc.scalar.activation(og_s, ps_a[:, 0:H], Act.Relu, scale=gsn[:, t, 8:9])
                    nc.scalar.activation(sh[:, 0:512], ps_a[:], Act.Relu)
                    nc.scalar.activation(sh[:, 512:1024], ps_b[:], Act.Relu)

                    for d in range(D_NUM):
                        ps = pb.tile([128, 512], f32, tag="pb", name=f"ps{g}_{t}_{d}")
                        nc.tensor.matmul(ps[:], ones_sb[:, :], bias_sb[:, 512 * d : 512 * d + 512],
                                         start=True, stop=False)
                        for c in range(KC):
                            nc.tensor.matmul(ps[:], xtg[:, d, c, t * 128 : (t + 1) * 128],
                                             wsp_sb[:, d, c, :], start=False, stop=(c == KC - 1))
                        nc.scalar.activation(spec[:, d, :], ps[:], Act.Relu)
                        og_d = og[:, d, t, :]
                        # first term: g2[0] * relu(spec expert 0), fused in the eviction
                        nc.scalar.activation(og_d, ps[:, 0:H], Act.Relu, scale=g2d[d][:, t, 0:1])
                        nc.vector.scalar_tensor_tensor(
                            og_d, spec[:, d, H : 2 * H], g2d[d][:, t, 1:2], og_d,
                            Alu.mult, Alu.add,
                        )
                        A = allowed[d]
                        if len(A) == 1 or not USE_GPS:
                            for s in A:
                                nc.vector.scalar_tensor_tensor(
                                    og_d, sh[:, s * H : (s + 1) * H],
                                    g2d[d][:, t, 2 + s : 3 + s], og_d, Alu.mult, Alu.add,
                                )
                        else:
                            accB = ap_.tile([128, H], f32, tag="accB", name=f"accB{g}_{t}_{d}")
                            s0, s1_ = A[0], A[1]
                            nc.gpsimd.tensor_scalar_mul(
                                accB[:], sh[:, s0 * H : (s0 + 1) * H], g2d[d][:, t, 2 + s0 : 3 + s0]
                            )
                            nc.gpsimd.scalar_tensor_tensor(
                                accB[:], sh[:, s1_ * H : (s1_ + 1) * H],
                                g2d[d][:, t, 2 + s1_ : 3 + s1_], accB[:], Alu.mult, Alu.add,
                            )
                            nc.vector.tensor_tensor(og_d, og_d, accB[:], Alu.add)

                    # out_sh tail: 8 spec terms + shared 1..3
                    if USE_GPS:
                        for d, e in ((0, 0), (0, 1), (1, 0), (1, 1)):
                            nc.vector.scalar_tensor_tensor(
                                og_s, spec[:, d, e * H : (e + 1) * H],
                                gsn[:, t, 2 * d + e : 2 * d + e + 1], og_s, Alu.mult, Alu.add,
                            )
                        accC = ap_.tile([128, H], f32, tag="accC", name=f"accC{g}_{t}")
                        nc.gpsimd.tensor_scalar_mul(accC[:], spec[:, 2, 0:H], gsn[:, t, 4:5])
                        for d, e in ((2, 1), (3, 0), (3, 1)):
                            nc.gpsimd.scalar_tensor_tensor(
                                accC[:], spec[:, d, e * H : (e + 1) * H],
                                gsn[:, t, 2 * d + e : 2 * d + e + 1], accC[:], Alu.mult, Alu.add,
                            )
                        for s in (1, 2, 3):
                            nc.gpsimd.scalar_tensor_tensor(
                                accC[:], sh[:, s * H : (s + 1) * H],
                                gsn[:, t, 8 + s : 9 + s], accC[:], Alu.mult, Alu.add,
                            )
                        nc.vector.tensor_tensor(og_s, og_s, accC[:], Alu.add)
                    else:
                        for d in range(D_NUM):
                            for e in range(N_ES):
                                nc.vector.scalar_tensor_tensor(
                                    og_s, spec[:, d, e * H : (e + 1) * H],
                                    gsn[:, t, 2 * d + e : 2 * d + e + 1], og_s, Alu.mult, Alu.add,
                                )
                        for s in (1, 2, 3):
                            nc.vector.scalar_tensor_tensor(
                                og_s, sh[:, s * H : (s + 1) * H],
                                gsn[:, t, 8 + s : 9 + s], og_s, Alu.mult, Alu.add,
                            )

                for t in range(GRP):
                    r0 = j0 + t * 128
                    nc.scalar.dma_start(
                        out[:, r0 : r0 + 128, :].rearrange("i p h -> p i h"),
                        og[:, :, t, :],
                    )

    nc.compile()
    return nc


def _prep_inputs(inputs):
    """Host-side shard + relayout. Returns (in_maps, allowed)."""
    x_list = np.asarray(inputs["x_list"], dtype=np.float32)
    sim_domain = np.asarray(inputs["sim_domain"])
    W_spec = np.asarray(inputs["W_spec"], dtype=np.float32)
    b_spec = np.asarray(inputs["b_spec"], dtype=np.float32)
    W_sh = np.asarray(inputs["W_sh"], dtype=np.float32)
    b_sh = np.asarray(inputs["b_sh"], dtype=np.float32)
    W_gate = np.asarray(inputs["W_gate"], dtype=np.float32)
    b_gate = np.asarray(inputs["b_gate"], dtype=np.float32)
    W_gate_sh = np.asarray(inputs["W_gate_sh"], dtype=np.float32)
    b_gate_sh = np.asarray(inputs["b_gate_sh"], dtype=np.float32)

    mem = (sim_domain[:, :, None] == np.arange(D_NUM)[None, None, :]).any(axis=1)  # [D, D]
    allowed = tuple(tuple(int(s) for s in range(N_SH) if mem[d, s]) for d in range(D_NUM))

    wsp = np.ascontiguousarray(W_spec.transpose(0, 2, 1, 3).reshape(D_NUM, DIN, N_ES * H))
    wsh = np.ascontiguousarray(W_sh.transpose(1, 0, 2).reshape(DIN, N_SH * H))
    wg = np.ascontiguousarray(W_gate.transpose(1, 0, 2).reshape(DIN, D_NUM * 6))
    wgs = np.ascontiguousarray(W_gate_sh)

    bias_row = np.concatenate(
        [b_spec.reshape(D_NUM * N_ES * H), b_sh.reshape(N_SH * H),
         b_gate.reshape(-1), b_gate_sh.reshape(-1)]
    ).astype(np.float32)
    assert bias_row.shape[0] == NB
    bias = np.zeros((32, NB), np.float32)
    bias[0] = bias_row
    ones = np.zeros((32, 128), np.float32)
    ones[0] = 1.0

    bmask_row = np.ones((D_NUM, 6), np.float32)
    bmask_row[:, N_ES:] = mem.astype(np.float32)
    bmask = np.broadcast_to(
        np.repeat(bmask_row[None, :, None, :], GRP, axis=2), (128, D_NUM, GRP, 6)
    ).copy()

    shared = {"wsp": wsp, "wsh": wsh, "wg": wg, "wgs": wgs,
              "bias": bias, "ones": ones, "bmask": bmask}
    in_maps = []
    for c in range(N_CORES):
        sl = x_list[:, c * BC : (c + 1) * BC, :]           # [5, BC, DIN]
        xt_c = np.ascontiguousarray(sl.transpose(0, 2, 1))  # [5, DIN, BC]
        in_maps.append({"xt": xt_c, **shared})
    return in_maps, allowed


def _run(inputs, trace=False, trace_kwargs=None):
    from concourse.bass_utils import run_bass_kernel_spmd

    in_maps, allowed = _prep_inputs(inputs)
    key = allowed
    if key not in _BUILD_CACHE:
        _BUILD_CACHE[key] = _build(allowed)
    nc = _BUILD_CACHE[key]

    kw = {}
    if trace:
        kw["trace"] = True
        if trace_kwargs:
            kw.update(trace_kwargs)
    res = run_bass_kernel_spmd(nc, in_maps, list(range(N_CORES)), **kw)
    full = np.empty((5, B, H), np.float32)
    for c in range(N_CORES):
        full[:, c * BC : (c + 1) * BC, :] = res.results[c]["out"]
    return full, res


def kernel(**inputs):
    full, _ = _run(inputs)
    return full


# revision 4
# speedup vs baseline: 2.9814x; 2.9814x over previous
"""Trainium2 Bass kernel for the CGC (Customized Gate Control) MoE routing module.

Contract: kernel(**inputs) takes the FULL unsharded inputs (numpy/jax arrays)
and returns the FULL output [5, 16384, 256] float32.

Strategy:
  - Data-parallel over batch across 8 NeuronCores (2048 rows/core).
  - Host-side prep: per-core x slices are fed pre-transposed [DIN, B_c] so the
    contraction dim lands on SBUF partitions with fully contiguous DMAs (no
    on-device transposes). Expert weights are replicated and packed [DIN, E*H].
  - Matmul operands are cast to bf16 (fp32 matmul is a 2-pass HI/LO operation
    on the TRN2 PE — half throughput); PSUM accumulation stays fp32.
  - Expert biases are accumulated into PSUM by a K=32 matmul against a one-hot
    "ones" row, so the ScalarE eviction is a pure relu (and folds the first
    gate-scaled combine term via its per-partition scale operand).
  - The double-softmax gate mask is known from sim_domain at trace time, so the
    kernel is compile-specialized: masked shared-expert terms are not emitted.
  - Combine chains run with UNNORMALIZED second-softmax numerators; the 1/sum
    factor is applied once per output tile at the end (per-partition scale).
"""

import sys

sys.path.insert(0, "/opt/trn_rl_repo")

import numpy as np

D_NUM = 4
N_ES = 2
N_SH = 4
DIN = 512
H = 256
B = 16384
N_CORES = 8
BC = B // N_CORES          # 2048 rows per core
KC = DIN // 128            # 4 contraction chunks
GRP = 4                    # batch tiles (of 128 rows) per group
NG = BC // (128 * GRP)     # groups per core

# bias row layout: [spec d*512 .. | shared 1024 | gate 4*6 | gate_sh 12]
OFF_SH = D_NUM * 512       # 2048
OFF_G = OFF_SH + N_SH * H  # 3072
OFF_GS = OFF_G + D_NUM * (N_ES + N_SH)  # 3096
NB = OFF_GS + D_NUM * N_ES + N_SH       # 3108

_BUILD_CACHE = {}


def _build(allowed):
    """Trace + compile the per-core kernel, specialized on the allowed
    shared-expert sets (from sim_domain). Returns the compiled Bacc."""
    import concourse.bacc as bacc
    import concourse.bass as bass
    import concourse.mybir as mybir
    import concourse.tile as tile

    f32 = mybir.dt.float32
    bf16 = mybir.dt.bfloat16
    Alu = mybir.AluOpType
    Act = mybir.ActivationFunctionType
    Ax = mybir.AxisListType

    nc = bacc.Bacc(None, target_bir_lowering=False, debug=False)

    xt = nc.declare_dram_parameter("xt", [5, DIN, BC], bf16, isOutput=False)
    wsp = nc.declare_dram_parameter("wsp", [D_NUM, DIN, N_ES * H], bf16, isOutput=False)
    wsh = nc.declare_dram_parameter("wsh", [DIN, N_SH * H], bf16, isOutput=False)
    wg = nc.declare_dram_parameter("wg", [DIN, D_NUM * 6], bf16, isOutput=False)
    wgs = nc.declare_dram_parameter("wgs", [DIN, 12], bf16, isOutput=False)
    bias = nc.declare_dram_parameter("bias", [32, NB], bf16, isOutput=False)
    ones = nc.declare_dram_parameter("ones", [32, 128], bf16, isOutput=False)
    bmask = nc.declare_dram_parameter("bmask", [128, D_NUM, GRP, 6], f32, isOutput=False)
    out = nc.declare_dram_parameter("out", [5, BC, H], f32, isOutput=True)

    with tile.TileContext(nc) as tc:
        with (
            tc.tile_pool(name="wpool", bufs=1) as wp,
            tc.tile_pool(name="xpool", bufs=2) as xp,
            tc.tile_pool(name="ogpool", bufs=2) as ogp,
            tc.tile_pool(name="specpool", bufs=2) as spp,
            tc.tile_pool(name="shpool", bufs=2) as shp,
            tc.tile_pool(name="smpool", bufs=3) as sp,
            tc.tile_pool(name="pbig", bufs=3, space=bass.MemorySpace.PSUM) as pb,
            tc.tile_pool(name="pgate", bufs=5, space=bass.MemorySpace.PSUM) as pg,
        ):
            # ---- persistent weights ----
            wsp_sb = wp.tile([128, D_NUM, KC, N_ES * H], bf16, tag="wsp")
            nc.sync.dma_start(wsp_sb[:], wsp.rearrange("d (c p) n -> p d c n", p=128))
            wsh_sb = wp.tile([128, KC, N_SH * H], bf16, tag="wsh")
            nc.sync.dma_start(wsh_sb[:], wsh.rearrange("(c p) n -> p c n", p=128))
            wg_sb = wp.tile([128, KC, D_NUM * 6], bf16, tag="wg")
            nc.sync.dma_start(wg_sb[:], wg.rearrange("(c p) n -> p c n", p=128))
            wgs_sb = wp.tile([128, KC, 12], bf16, tag="wgs")
            nc.sync.dma_start(wgs_sb[:], wgs.rearrange("(c p) n -> p c n", p=128))
            bias_sb = wp.tile([32, NB], bf16, tag="bias")
            nc.sync.dma_start(bias_sb[:], bias[:])
            ones_sb = wp.tile([32, 128], bf16, tag="ones")
            nc.sync.dma_start(ones_sb[:], ones[:])
            bmask_sb = wp.tile([128, D_NUM, GRP, 6], f32, tag="bmask")
            nc.sync.dma_start(bmask_sb[:], bmask[:])

            for g in range(NG):
                j0 = g * (GRP * 128)
                xtg = xp.tile([128, 5, KC, GRP * 128], bf16, tag="xtg", name=f"xtg{g}")
                nc.sync.dma_start(
                    xtg[:],
                    xt[:, :, j0 : j0 + GRP * 128].rearrange("i (c p) j -> p i c j", p=128),
                )
                og = ogp.tile([128, 5, GRP, H], bf16, tag="og", name=f"og{g}")

                # ---- gate phase (PE): all gate logits for the group ----
                pgd = []
                for d in range(D_NUM):
                    pgt = pg.tile([128, GRP, 6], f32, tag="pg", name=f"pg{g}_{d}")
                    pgd.append(pgt)
                    for t in range(GRP):
                        nc.tensor.matmul(
                            pgt[:, t, :], ones_sb[:, :],
                            bias_sb[:, OFF_G + 6 * d : OFF_G + 6 * d + 6],
                            start=True, stop=False,
                        )
                        for c in range(KC):
                            nc.tensor.matmul(
                                pgt[:, t, :],
                                xtg[:, d, c, t * 128 : (t + 1) * 128],
                                wg_sb[:, c, 6 * d : 6 * d + 6],
                                start=False, stop=(c == KC - 1),
                            )
                pgs = pg.tile([128, GRP, 12], f32, tag="pg", name=f"pgs{g}")
                for t in range(GRP):
                    nc.tensor.matmul(
                        pgs[:, t, :], ones_sb[:, :], bias_sb[:, OFF_GS : OFF_GS + 12],
                        start=True, stop=False,
                    )
                    for c in range(KC):
                        nc.tensor.matmul(
                            pgs[:, t, :],
                            xtg[:, 4, c, t * 128 : (t + 1) * 128],
                            wgs_sb[:, c, :],
                            start=False, stop=(c == KC - 1),
                        )

                # ---- softmax phase (ACT exp + batched DVE reductions) ----
                e1 = sp.tile([128, D_NUM, GRP, 6], f32, tag="e1", name=f"e1_{g}")
                for d in range(D_NUM):
                    nc.scalar.activation(e1[:, d], pgd[d][:], Act.Exp)
                s1 = sp.tile([128, D_NUM, GRP], f32, tag="s1", name=f"s1_{g}")
                nc.vector.tensor_reduce(s1[:], e1[:], axis=Ax.X, op=Alu.add)
                r1 = sp.tile([128, D_NUM, GRP], f32, tag="r1", name=f"r1_{g}")
                nc.vector.reciprocal(r1[:], s1[:])
                # e2 = exp(softmax1) — the scale operand applies 1/s1 inside the exp
                e2 = sp.tile([128, D_NUM, GRP, 6], f32, tag="e2", name=f"e2_{g}")
                for d in range(D_NUM):
                    for t in range(GRP):
                        nc.scalar.activation(
                            e2[:, d, t, :], e1[:, d, t, :], Act.Exp,
                            scale=r1[:, d, t : t + 1],
                        )
                e2m = sp.tile([128, D_NUM, GRP, 6], f32, tag="e2m", name=f"e2m_{g}")
                nc.vector.tensor_tensor(e2m[:], e2[:], bmask_sb[:], Alu.mult)
                s2 = sp.tile([128, D_NUM, GRP], f32, tag="s2", name=f"s2_{g}")
                nc.vector.tensor_reduce(s2[:], e2m[:], axis=Ax.X, op=Alu.add)
                r2 = sp.tile([128, D_NUM, GRP], f32, tag="r2", name=f"r2_{g}")
                nc.vector.reciprocal(r2[:], s2[:])

                egs = sp.tile([128, GRP, 12], f32, tag="egs", name=f"egs{g}")
                nc.scalar.activation(egs[:], pgs[:], Act.Exp)
                sgs = sp.tile([128, GRP], f32, tag="sgs", name=f"sgs{g}")
                nc.vector.tensor_reduce(sgs[:], egs[:], axis=Ax.X, op=Alu.add)
                rgs = sp.tile([128, GRP], f32, tag="rgs", name=f"rgs{g}")
                nc.vector.reciprocal(rgs[:], sgs[:])

                # ---- expert + combine phase, per batch tile ----
                for t in range(GRP):
                    spec = spp.tile([128, D_NUM, N_ES * H], bf16, tag="spec", name=f"spec{g}_{t}")
                    sh = shp.tile([128, N_SH * H], bf16, tag="sh", name=f"sh{g}_{t}")

                    # shared experts (2 PSUM banks)
                    ps_a = pb.tile([128, 512], f32, tag="pb", name=f"psa{g}_{t}")
                    nc.tensor.matmul(ps_a[:], ones_sb[:, :], bias_sb[:, OFF_SH : OFF_SH + 512],
                                     start=True, stop=False)
                    for c in range(KC):
                        nc.tensor.matmul(ps_a[:], xtg[:, 4, c, t * 128 : (t + 1) * 128],
                                         wsh_sb[:, c, 0:512], start=False, stop=(c == KC - 1))
                    ps_b = pb.tile([128, 512], f32, tag="pb", name=f"psb{g}_{t}")
                    nc.tensor.matmul(ps_b[:], ones_sb[:, :], bias_sb[:, OFF_SH + 512 : OFF_SH + 1024],
                                     start=True, stop=False)
                    for c in range(KC):
                        nc.tensor.matmul(ps_b[:], xtg[:, 4, c, t * 128 : (t + 1) * 128],
                                         wsh_sb[:, c, 512:1024], start=False, stop=(c == KC - 1))

                    og_s = og[:, 4, t, :]
                    # out_sh first term: egs[8] * relu(shared expert 0)  (unnormalized)
                    nc.scalar.activation(og_s, ps_a[:, 0:H], Act.Relu, scale=egs[:, t, 8:9])
                    nc.scalar.activation(sh[:, 0:512], ps_a[:], Act.Relu)
                    nc.scalar.activation(sh[:, 512:1024], ps_b[:], Act.Relu)

                    for d in range(D_NUM):
                        ps = pb.tile([128, 512], f32, tag="pb", name=f"ps{g}_{t}_{d}")
                        nc.tensor.matmul(ps[:], ones_sb[:, :], bias_sb[:, 512 * d : 512 * d + 512],
                                         start=True, stop=False)
                        for c in range(KC):
                            nc.tensor.matmul(ps[:], xtg[:, d, c, t * 128 : (t + 1) * 128],
                                             wsp_sb[:, d, c, :], start=False, stop=(c == KC - 1))
                        nc.scalar.activation(spec[:, d, :], ps[:], Act.Relu)
                        og_d = og[:, d, t, :]
                        # first term: e2m[0] * relu(spec expert 0), fused in the eviction
                        nc.scalar.activation(og_d, ps[:, 0:H], Act.Relu,
                                             scale=e2m[:, d, t, 0:1])
                        nc.vector.scalar_tensor_tensor(
                            og_d, spec[:, d, H : 2 * H], e2m[:, d, t, 1:2], og_d,
                            Alu.mult, Alu.add,
                        )
                        for s in allowed[d]:
                            nc.vector.scalar_tensor_tensor(
                                og_d, sh[:, s * H : (s + 1) * H],
                                e2m[:, d, t, 2 + s : 3 + s], og_d, Alu.mult, Alu.add,
                            )
                        # normalize by 1/sum(e2m) once
                        nc.vector.tensor_scalar_mul(og_d, og_d, r2[:, d, t : t + 1])

                    # out_sh tail: 8 spec terms + shared 1..3, then normalize
                    for d in range(D_NUM):
                        for e in range(N_ES):
                            nc.vector.scalar_tensor_tensor(
                                og_s, spec[:, d, e * H : (e + 1) * H],
                                egs[:, t, 2 * d + e : 2 * d + e + 1], og_s, Alu.mult, Alu.add,
                            )
                    for s in (1, 2, 3):
                        nc.vector.scalar_tensor_tensor(
                            og_s, sh[:, s * H : (s + 1) * H],
                            egs[:, t, 8 + s : 9 + s], og_s, Alu.mult, Alu.add,
                        )
                    nc.vector.tensor_scalar_mul(og_s, og_s, rgs[:, t : t + 1])

                for t in range(GRP):
                    r0 = j0 + t * 128
                    nc.gpsimd.dma_start(
                        out[:, r0 : r0 + 128, :].rearrange("i p h -> p i h"),
                        og[:, :, t, :],
                    )

    nc.compile()
    return nc


def _prep_inputs(inputs):
    """Host-side shard + relayout. Returns (in_maps, allowed)."""
    try:
        import ml_dtypes
        bf16_np = ml_dtypes.bfloat16
    except ImportError:  # pragma: no cover
        import jax.numpy as jnp
        bf16_np = jnp.bfloat16

    x_list = np.asarray(inputs["x_list"], dtype=np.float32)
    sim_domain = np.asarray(inputs["sim_domain"])
    W_spec = np.asarray(inputs["W_spec"], dtype=np.float32)
    b_spec = np.asarray(inputs["b_spec"], dtype=np.float32)
    W_sh = np.asarray(inputs["W_sh"], dtype=np.float32)
    b_sh = np.asarray(inputs["b_sh"], dtype=np.float32)
    W_gate = np.asarray(inputs["W_gate"], dtype=np.float32)
    b_gate = np.asarray(inputs["b_gate"], dtype=np.float32)
    W_gate_sh = np.asarray(inputs["W_gate_sh"], dtype=np.float32)
    b_gate_sh = np.asarray(inputs["b_gate_sh"], dtype=np.float32)

    mem = (sim_domain[:, :, None] == np.arange(D_NUM)[None, None, :]).any(axis=1)  # [D, D]
    allowed = tuple(tuple(int(s) for s in range(N_SH) if mem[d, s]) for d in range(D_NUM))

    wsp = np.ascontiguousarray(
        W_spec.transpose(0, 2, 1, 3).reshape(D_NUM, DIN, N_ES * H)
    ).astype(bf16_np)
    wsh = np.ascontiguousarray(W_sh.transpose(1, 0, 2).reshape(DIN, N_SH * H)).astype(bf16_np)
    wg = np.ascontiguousarray(W_gate.transpose(1, 0, 2).reshape(DIN, D_NUM * 6)).astype(bf16_np)
    wgs = np.ascontiguousarray(W_gate_sh).astype(bf16_np)

    bias_row = np.concatenate(
        [b_spec.reshape(D_NUM * N_ES * H), b_sh.reshape(N_SH * H),
         b_gate.reshape(-1), b_gate_sh.reshape(-1)]
    ).astype(np.float32)
    assert bias_row.shape[0] == NB
    bias = np.zeros((32, NB), np.float32)
    bias[0] = bias_row
    bias = bias.astype(bf16_np)
    ones = np.zeros((32, 128), np.float32)
    ones[0] = 1.0
    ones = ones.astype(bf16_np)

    bmask_row = np.ones((D_NUM, 6), np.float32)
    bmask_row[:, N_ES:] = mem.astype(np.float32)
    bmask = np.broadcast_to(
        np.repeat(bmask_row[None, :, None, :], GRP, axis=2), (128, D_NUM, GRP, 6)
    ).copy()

    shared = {"wsp": wsp, "wsh": wsh, "wg": wg, "wgs": wgs,
              "bias": bias, "ones": ones, "bmask": bmask}
    in_maps = []
    for c in range(N_CORES):
        sl = x_list[:, c * BC : (c + 1) * BC, :]
        xt_c = np.ascontiguousarray(sl.transpose(0, 2, 1)).astype(bf16_np)  # [5, DIN, BC]
        in_maps.append({"xt": xt_c, **shared})
    return in_maps, allowed


def _run(inputs, trace=False, trace_kwargs=None):
    from concourse.bass_utils import run_bass_kernel_spmd

    in_maps, allowed = _prep_inputs(inputs)
    key = allowed
    if key not in _BUILD_CACHE:
        _BUILD_CACHE[key] = _build(allowed)
    nc = _BUILD_CACHE[key]

    kw = {}
    if trace:
        kw["trace"] = True
        if trace_kwargs:
            kw.update(trace_kwargs)
    res = run_bass_kernel_spmd(nc, in_maps, list(range(N_CORES)), **kw)
    full = np.empty((5, B, H), np.float32)
    for c in range(N_CORES):
        full[:, c * BC : (c + 1) * BC, :] = res.results[c]["out"]
    return full, res


def kernel(**inputs):
    full, _ = _run(inputs)
    return full


# revision 5
# speedup vs baseline: 3.4645x; 1.1620x over previous
"""Trainium2 Bass kernel for the CGC (Customized Gate Control) MoE routing module.

Contract: kernel(**inputs) takes the FULL unsharded inputs (numpy/jax arrays)
and returns the FULL output [5, 16384, 256] float32.

Strategy:
  - Data-parallel over batch across 8 NeuronCores (2048 rows/core).
  - Host prep: per-core x slices fed pre-transposed [DIN, B_c] (contraction dim
    on SBUF partitions, fully contiguous DMAs, no on-device transposes);
    weights replicated, packed [DIN, E*H], cast to bf16 (fp32 matmul is a
    2-pass HI/LO op on the TRN2 PE - half throughput). PSUM stays fp32.
  - Expert biases enter PSUM via a K=32 one-hot matmul, so downstream consumers
    see z+b directly.
  - All gate logits for a group live in ONE PSUM bank (one bias matmul, then
    region-wise accumulation), softmax runs batched over 4 domains x 8 tiles
    with broadcast (step-0) tensor_tensor ops for the normalizations.
  - The gated combine uses a runtime-registered custom DVE op RELU_MAC:
        out = max(in0*s0 + in1, in1)  ==  s0*relu(in0) + in1   (s0 >= 0)
    reading expert PSUM banks directly - no relu evictions to SBUF at all.
    First terms are fused into ScalarE scaled-relu evictions; the out_sh
    shared-expert adds run on GpSimd to offload the Vector engine.
  - The double-softmax mask is known from sim_domain at trace time; masked
    shared-expert terms are not emitted (kernel is compile-specialized).
"""

import sys

sys.path.insert(0, "/opt/trn_rl_repo")

import numpy as np

D_NUM = 4
N_ES = 2
N_SH = 4
DIN = 512
H = 256
B = 16384
N_CORES = 8
BC = B // N_CORES          # 2048 rows per core
KC = DIN // 128            # 4 contraction chunks
GRP = 8                    # batch tiles (of 128 rows) per group
NG = BC // (128 * GRP)     # groups per core

# bias row layout: [spec d*512 | shared 1024 | (unused 36) | gate-bank 288]
OFF_SH = D_NUM * 512                     # 2048
OFF_GB = OFF_SH + N_SH * H + 36          # 3108 gate-bank bias region
GB_GS = D_NUM * GRP * 6                  # gsh region offset inside gate bank
NGB = GB_GS + GRP * 12                   # 288
NB = OFF_GB + NGB                        # 3396

_BUILD_CACHE = {}
_RELU_MAC = None


def _get_relu_mac():
    """Register the RELU_MAC custom DVE op (idempotent)."""
    global _RELU_MAC
    if _RELU_MAC is not None:
        return _RELU_MAC
    from concourse import dve_ops
    from concourse.dve_spec import Spec, Src0, Src1, C0, maxx, lower, _has_src1
    from concourse.dve_uop import DveOpSpec

    name = "RELU_MAC_ANT"
    for o in dve_ops.OPS:
        if o.name == name:
            _RELU_MAC = o
            return o
    spec = Spec(
        body=maxx(Src0 * C0 + Src1, Src1),
        reference=lambda in0, in1, s0, s1, imm2: np.maximum(
            in0.astype(np.float32) * s0 + in1, in1
        ),
    )
    row = max(dve_ops._SUB_OPCODE_FOR_NAME.values()) + 1
    assert row < 0x20
    dve_ops._SUB_OPCODE_FOR_NAME[name] = row
    shas = {}
    for ver in ("v3", "v4"):
        tmp = DveOpSpec(name=name, opcode=row, uops=lower(spec, ver=ver),
                        rd1_en=_has_src1(spec))
        shas[ver] = tmp.sha(ver)
    op = dve_ops.DveOp(name, spec, subdim=False, uops_sha=shas)
    dve_ops.OPS.append(op)
    dve_ops.CUSTOM_DVE_SPECS[name] = spec
    _RELU_MAC = op
    return op


def _build(allowed):
    """Trace + compile the per-core kernel, specialized on the allowed
    shared-expert sets (from sim_domain)."""
    import concourse.bacc as bacc
    import concourse.bass as bass
    import concourse.mybir as mybir
    import concourse.tile as tile

    RELU_MAC = _get_relu_mac()

    f32 = mybir.dt.float32
    bf16 = mybir.dt.bfloat16
    Alu = mybir.AluOpType
    Act = mybir.ActivationFunctionType
    Ax = mybir.AxisListType

    nc = bacc.Bacc(None, target_bir_lowering=False, debug=False)

    xt = nc.declare_dram_parameter("xt", [5, DIN, BC], bf16, isOutput=False)
    wsp = nc.declare_dram_parameter("wsp", [D_NUM, DIN, N_ES * H], bf16, isOutput=False)
    wsh = nc.declare_dram_parameter("wsh", [DIN, N_SH * H], bf16, isOutput=False)
    wg = nc.declare_dram_parameter("wg", [DIN, D_NUM * 6], bf16, isOutput=False)
    wgs = nc.declare_dram_parameter("wgs", [DIN, 12], bf16, isOutput=False)
    bias = nc.declare_dram_parameter("bias", [32, NB], bf16, isOutput=False)
    ones = nc.declare_dram_parameter("ones", [32, 128], bf16, isOutput=False)
    bmask = nc.declare_dram_parameter("bmask", [128, D_NUM, GRP, 6], f32, isOutput=False)
    out = nc.declare_dram_parameter("out", [5, BC, H], f32, isOutput=True)

    with tile.TileContext(nc) as tc:
        with (
            tc.tile_pool(name="wpool", bufs=1) as wp,
            tc.tile_pool(name="xpool", bufs=2) as xp,
            tc.tile_pool(name="ogpool", bufs=2) as ogp,
            tc.tile_pool(name="smpool", bufs=3) as sp,
            tc.tile_pool(name="scrpool", bufs=10) as scp,
            tc.tile_pool(name="pbig", bufs=6, space=bass.MemorySpace.PSUM) as pb,
            tc.tile_pool(name="pgate", bufs=2, space=bass.MemorySpace.PSUM) as pg,
        ):
            # ---- persistent weights ----
            wsp_sb = wp.tile([128, D_NUM, KC, N_ES * H], bf16, tag="wsp")
            nc.sync.dma_start(wsp_sb[:], wsp.rearrange("d (c p) n -> p d c n", p=128))
            wsh_sb = wp.tile([128, KC, N_SH * H], bf16, tag="wsh")
            nc.sync.dma_start(wsh_sb[:], wsh.rearrange("(c p) n -> p c n", p=128))
            wg_sb = wp.tile([128, KC, D_NUM * 6], bf16, tag="wg")
            nc.sync.dma_start(wg_sb[:], wg.rearrange("(c p) n -> p c n", p=128))
            wgs_sb = wp.tile([128, KC, 12], bf16, tag="wgs")
            nc.sync.dma_start(wgs_sb[:], wgs.rearrange("(c p) n -> p c n", p=128))
            bias_sb = wp.tile([32, NB], bf16, tag="bias")
            nc.sync.dma_start(bias_sb[:], bias[:])
            ones_sb = wp.tile([32, 128], bf16, tag="ones")
            nc.sync.dma_start(ones_sb[:], ones[:])
            bmask_sb = wp.tile([128, D_NUM, GRP, 6], f32, tag="bmask")
            nc.sync.dma_start(bmask_sb[:], bmask[:])

            for g in range(NG):
                j0 = g * (GRP * 128)
                xtg = xp.tile([128, 5, KC, GRP * 128], bf16, tag="xtg", name=f"xtg{g}")
                nc.sync.dma_start(
                    xtg[:],
                    xt[:, :, j0 : j0 + GRP * 128].rearrange("i (c p) j -> p i c j", p=128),
                )
                og = ogp.tile([128, 5, GRP, H], bf16, tag="og", name=f"og{g}")

                # ---- gate phase: one PSUM bank holds every gate logit ----
                gbank = pg.tile([128, NGB], f32, tag="pg", name=f"gb{g}")
                nc.tensor.matmul(gbank[:], ones_sb[:, :], bias_sb[:, OFF_GB:NB],
                                 start=True, stop=False, skip_group_check=True)
                for d in range(D_NUM):
                    for t in range(GRP):
                        o0 = (d * GRP + t) * 6
                        for c in range(KC):
                            nc.tensor.matmul(
                                gbank[:, o0 : o0 + 6],
                                xtg[:, d, c, t * 128 : (t + 1) * 128],
                                wg_sb[:, c, 6 * d : 6 * d + 6],
                                start=False, stop=False, skip_group_check=True,
                            )
                for t in range(GRP):
                    o0 = GB_GS + t * 12
                    for c in range(KC):
                        nc.tensor.matmul(
                            gbank[:, o0 : o0 + 12],
                            xtg[:, 4, c, t * 128 : (t + 1) * 128],
                            wgs_sb[:, c, :],
                            start=False,
                            stop=(t == GRP - 1 and c == KC - 1),
                            skip_group_check=True,
                        )

                # ---- batched softmax phase ----
                gview = gbank[:, 0:GB_GS].rearrange("p (d t s) -> p d t s", d=D_NUM, t=GRP)
                e1 = sp.tile([128, D_NUM, GRP, 6], f32, tag="e1", name=f"e1_{g}")
                nc.scalar.activation(e1[:], gview, Act.Exp)
                s1 = sp.tile([128, D_NUM, GRP], f32, tag="s1", name=f"s1_{g}")
                nc.vector.tensor_reduce(s1[:], e1[:], axis=Ax.X, op=Alu.add)
                r1 = sp.tile([128, D_NUM, GRP], f32, tag="r1", name=f"r1_{g}")
                nc.vector.reciprocal(r1[:], s1[:])
                gn = sp.tile([128, D_NUM, GRP, 6], f32, tag="gn", name=f"gn_{g}")
                nc.vector.tensor_tensor(
                    gn[:], e1[:], r1[:, :, :, None].to_broadcast([128, D_NUM, GRP, 6]),
                    Alu.mult,
                )
                e2 = sp.tile([128, D_NUM, GRP, 6], f32, tag="e2", name=f"e2_{g}")
                nc.scalar.activation(e2[:], gn[:], Act.Exp)
                e2m = sp.tile([128, D_NUM, GRP, 6], f32, tag="e2m", name=f"e2m_{g}")
                nc.vector.tensor_tensor(e2m[:], e2[:], bmask_sb[:], Alu.mult)
                s2 = sp.tile([128, D_NUM, GRP], f32, tag="s2", name=f"s2_{g}")
                nc.vector.tensor_reduce(s2[:], e2m[:], axis=Ax.X, op=Alu.add)
                r2 = sp.tile([128, D_NUM, GRP], f32, tag="r2", name=f"r2_{g}")
                nc.vector.reciprocal(r2[:], s2[:])
                g2 = sp.tile([128, D_NUM, GRP, 6], f32, tag="g2", name=f"g2_{g}")
                nc.vector.tensor_tensor(
                    g2[:], e2m[:], r2[:, :, :, None].to_broadcast([128, D_NUM, GRP, 6]),
                    Alu.mult,
                )

                gsview = gbank[:, GB_GS:NGB].rearrange("p (t s) -> p t s", t=GRP)
                egs = sp.tile([128, GRP, 12], f32, tag="egs", name=f"egs{g}")
                nc.scalar.activation(egs[:], gsview, Act.Exp)
                sgs = sp.tile([128, GRP], f32, tag="sgs", name=f"sgs{g}")
                nc.vector.tensor_reduce(sgs[:], egs[:], axis=Ax.X, op=Alu.add)
                rgs = sp.tile([128, GRP], f32, tag="rgs", name=f"rgs{g}")
                nc.vector.reciprocal(rgs[:], sgs[:])
                gs = sp.tile([128, GRP, 12], f32, tag="gs", name=f"gs{g}")
                nc.vector.tensor_tensor(
                    gs[:], egs[:], rgs[:, :, None].to_broadcast([128, GRP, 12]), Alu.mult
                )

                # ---- expert + combine phase, per batch tile ----
                for t in range(GRP):
                    og_s = og[:, 4, t, :]
                    psd = []
                    for d in range(D_NUM):
                        ps = pb.tile([128, 512], f32, tag="pb", name=f"ps{g}_{t}_{d}")
                        psd.append(ps)
                        nc.tensor.matmul(ps[:], ones_sb[:, :],
                                         bias_sb[:, 512 * d : 512 * d + 512],
                                         start=True, stop=False)
                        for c in range(KC):
                            nc.tensor.matmul(ps[:], xtg[:, d, c, t * 128 : (t + 1) * 128],
                                             wsp_sb[:, d, c, :], start=False,
                                             stop=(c == KC - 1))
                    for d in range(D_NUM):
                        ps = psd[d]
                        og_d = og[:, d, t, :]
                        # outs[d] spec terms: ScalarE seed + one RELU_MAC
                        nc.scalar.activation(og_d, ps[:, 0:H], Act.Relu,
                                             scale=g2[:, d, t, 0:1])
                        nc.vector._custom_dve(RELU_MAC, out=og_d, in0=ps[:, H : 2 * H],
                                              in1=og_d, s0=g2[:, d, t, 1:2])
                        # out_sh spec terms from the same PSUM banks
                        if d == 0:
                            nc.scalar.activation(og_s, ps[:, 0:H], Act.Relu,
                                                 scale=gs[:, t, 0:1])
                        else:
                            nc.vector._custom_dve(RELU_MAC, out=og_s, in0=ps[:, 0:H],
                                                  in1=og_s, s0=gs[:, t, 2 * d : 2 * d + 1])
                        nc.vector._custom_dve(RELU_MAC, out=og_s, in0=ps[:, H : 2 * H],
                                              in1=og_s, s0=gs[:, t, 2 * d + 1 : 2 * d + 2])

                    # shared experts
                    ps_a = pb.tile([128, 512], f32, tag="pb", name=f"psa{g}_{t}")
                    nc.tensor.matmul(ps_a[:], ones_sb[:, :],
                                     bias_sb[:, OFF_SH : OFF_SH + 512],
                                     start=True, stop=False)
                    for c in range(KC):
                        nc.tensor.matmul(ps_a[:], xtg[:, 4, c, t * 128 : (t + 1) * 128],
                                         wsh_sb[:, c, 0:512], start=False,
                                         stop=(c == KC - 1))
                    ps_b = pb.tile([128, 512], f32, tag="pb", name=f"psb{g}_{t}")
                    nc.tensor.matmul(ps_b[:], ones_sb[:, :],
                                     bias_sb[:, OFF_SH + 512 : OFF_SH + 1024],
                                     start=True, stop=False)
                    for c in range(KC):
                        nc.tensor.matmul(ps_b[:], xtg[:, 4, c, t * 128 : (t + 1) * 128],
                                         wsh_sb[:, c, 512:1024], start=False,
                                         stop=(c == KC - 1))
                    shp = {0: (ps_a, 0), 1: (ps_a, H), 2: (ps_b, 0), 3: (ps_b, H)}

                    # domain outputs: allowed shared-expert terms (RELU_MAC)
                    for d in range(D_NUM):
                        og_d = og[:, d, t, :]
                        for s in allowed[d]:
                            bank, off = shp[s]
                            nc.vector._custom_dve(
                                RELU_MAC, out=og_d, in0=bank[:, off : off + H],
                                in1=og_d, s0=g2[:, d, t, 2 + s : 3 + s],
                            )
                    # out_sh shared terms: ScalarE scaled-relu evict + GpSimd add
                    for s in range(N_SH):
                        bank, off = shp[s]
                        scr = scp.tile([128, H], bf16, tag="scr", name=f"scr{g}_{t}_{s}")
                        nc.scalar.activation(scr[:], bank[:, off : off + H], Act.Relu,
                                             scale=gs[:, t, 8 + s : 9 + s])
                        nc.gpsimd.tensor_tensor(og_s, og_s, scr[:], Alu.add)

                for t in range(GRP):
                    r0 = j0 + t * 128
                    nc.gpsimd.dma_start(
                        out[:, r0 : r0 + 128, :].rearrange("i p h -> p i h"),
                        og[:, :, t, :],
                    )

    nc.compile()
    return nc


def _prep_inputs(inputs):
    """Host-side shard + relayout. Returns (in_maps, allowed)."""
    import ml_dtypes
    bf16_np = ml_dtypes.bfloat16

    x_list = np.asarray(inputs["x_list"], dtype=np.float32)
    sim_domain = np.asarray(inputs["sim_domain"])
    W_spec = np.asarray(inputs["W_spec"], dtype=np.float32)
    b_spec = np.asarray(inputs["b_spec"], dtype=np.float32)
    W_sh = np.asarray(inputs["W_sh"], dtype=np.float32)
    b_sh = np.asarray(inputs["b_sh"], dtype=np.float32)
    W_gate = np.asarray(inputs["W_gate"], dtype=np.float32)
    b_gate = np.asarray(inputs["b_gate"], dtype=np.float32)
    W_gate_sh = np.asarray(inputs["W_gate_sh"], dtype=np.float32)
    b_gate_sh = np.asarray(inputs["b_gate_sh"], dtype=np.float32)

    mem = (sim_domain[:, :, None] == np.arange(D_NUM)[None, None, :]).any(axis=1)
    allowed = tuple(tuple(int(s) for s in range(N_SH) if mem[d, s]) for d in range(D_NUM))

    wsp = np.ascontiguousarray(
        W_spec.transpose(0, 2, 1, 3).reshape(D_NUM, DIN, N_ES * H)
    ).astype(bf16_np)
    wsh = np.ascontiguousarray(W_sh.transpose(1, 0, 2).reshape(DIN, N_SH * H)).astype(bf16_np)
    wg = np.ascontiguousarray(W_gate.transpose(1, 0, 2).reshape(DIN, D_NUM * 6)).astype(bf16_np)
    wgs = np.ascontiguousarray(W_gate_sh).astype(bf16_np)

    # gate-bank bias region: per (d, t) the domain gate bias, then per t gsh bias
    gb_bias = np.concatenate(
        [np.repeat(b_gate[:, None, :], GRP, axis=1).reshape(-1),
         np.tile(b_gate_sh, GRP)]
    )
    bias_row = np.concatenate(
        [b_spec.reshape(D_NUM * N_ES * H), b_sh.reshape(N_SH * H),
         np.zeros(36, np.float32), gb_bias]
    ).astype(np.float32)
    assert bias_row.shape[0] == NB
    bias = np.zeros((32, NB), np.float32)
    bias[0] = bias_row
    bias = bias.astype(bf16_np)
    ones = np.zeros((32, 128), np.float32)
    ones[0] = 1.0
    ones = ones.astype(bf16_np)

    bmask_row = np.ones((D_NUM, 6), np.float32)
    bmask_row[:, N_ES:] = mem.astype(np.float32)
    bmask = np.broadcast_to(
        np.repeat(bmask_row[None, :, None, :], GRP, axis=2), (128, D_NUM, GRP, 6)
    ).copy()

    shared = {"wsp": wsp, "wsh": wsh, "wg": wg, "wgs": wgs,
              "bias": bias, "ones": ones, "bmask": bmask}
    in_maps = []
    for c in range(N_CORES):
        sl = x_list[:, c * BC : (c + 1) * BC, :]
        xt_c = np.ascontiguousarray(sl.transpose(0, 2, 1)).astype(bf16_np)
        in_maps.append({"xt": xt_c, **shared})
    return in_maps, allowed


def _run(inputs, trace=False, trace_kwargs=None):
    from concourse.bass_utils import run_bass_kernel_spmd

    in_maps, allowed = _prep_inputs(inputs)
    key = allowed
    if key not in _BUILD_CACHE:
        _BUILD_CACHE[key] = _build(allowed)
    nc = _BUILD_CACHE[key]

    kw = {}
    if trace:
        kw["trace"] = True
        if trace_kwargs:
            kw.update(trace_kwargs)
    res = run_bass_kernel_spmd(nc, in_maps, list(range(N_CORES)), **kw)
    full = np.empty((5, B, H), np.float32)
    for c in range(N_CORES):
        full[:, c * BC : (c + 1) * BC, :] = res.results[c]["out"]
    return full, res


def kernel(**inputs):
    full, _ = _run(inputs)
    return full
